# revision 33
# baseline (speedup 1.0000x reference)
"""Trainium2 Bass kernel v2 for nn_Critic GNN message-passing critic.

Data-parallel: 128 graphs/core x 8 cores. All PE operands bf16 (fp32 matmul
costs 4 cyc/row on TRN2; bf16 costs 1). Per-edge endpoint features are
gathered on the HOST into gxE (x_src|x_dst|edge_attr|ones rows), so the edge
MLP's first layer is ONE 81-row stationary matmul per (8-graph group, 128-edge
chunk). W2 is folded through the scatter-add. The critic head runs per
4-graph subgroup: 1024 edge slots in a [128,1024] PSUM tile (512+512 relu
splits ACT/DVE) and the 3 factory slots in a shared per-pair tile feeding a
second finale matmul, so there is no padding anywhere.

PSUM budget (8 banks): tag A [128,512]f32 x3 + T [128,256]bf16 x1 +
P6 [128,1024]f32 x2  = 3 + 1 + 4.
"""

import numpy as np
import ml_dtypes
from contextlib import ExitStack

from concourse import bass, bacc, tile, bass_utils
from concourse import mybir

f32 = mybir.dt.float32
bf16 = mybir.dt.bfloat16
fp8 = mybir.dt.float8e4
BF = ml_dtypes.bfloat16
F8 = ml_dtypes.float8_e4m3
RELU = mybir.ActivationFunctionType.Relu
MAX = mybir.AluOpType.max
ADD = mybir.AluOpType.add
DR = mybir.MatmulPerfMode.DoubleRow

B, NN, NODE, EDGEF, HID, NFACT, NE = 1024, 64, 4, 2, 32, 3, 1024
NCORES = 8
GPC = B // NCORES          # 128 graphs per core
NPAIR = 8                  # pairs of 8-graph groups
E2 = NE + NFACT            # 1027

_CACHE = {}


def _build_nc(nrep=1, no_fact=False, no_strip32=False):
    nc = bacc.Bacc("TRN2", target_bir_lowering=False, debug=False,
                   num_devices=NCORES)

    def din(name, shape, dt=bf16):
        return nc.dram_tensor(name, shape, dt, kind="ExternalInput").ap()

    gxE = din("gxE", [81, 16 * NE])        # gathered x|ea|ones per 8g-group
    Wblk = din("Wblk", [81, 256])
    St = din("St", [128, 8, 64], fp8)
    idB = din("idB", [128, 64])
    xT2c = din("xT2c", [17, 32 * 64])
    Wla4c = din("Wla4c", [17, 128])
    Wlb4c = din("Wlb4c", [17, 128])
    WlapB = din("WlapB", [128, 128])
    WlbpB = din("WlbpB", [128, 128])
    G2t = din("G2t", [128, E2 + 13])
    selD = din("selD", [2, 2, 128], fp8)
    selF4 = din("selF4", [4, 128])
    selC8 = din("selC8", [8, 128])
    actD0 = din("actD0", [2, 2, 16 * NE], fp8)
    actD1 = din("actD1", [2, 2, 16 * NE], fp8)
    actF = din("actF", [8, 48])
    blc = din("blc", [128, 1], f32)
    WvP = din("WvP", [128, 4], f32)
    WvPb = din("WvPb", [128, 4])
    vout = nc.dram_tensor("v", [4, 160], f32, kind="ExternalOutput").ap()

    with tile.TileContext(nc) as tc:
        with ExitStack() as ctx:
            cpool = ctx.enter_context(tc.tile_pool(name="consts", bufs=1))
            accp = ctx.enter_context(tc.tile_pool(name="accp", bufs=2))
            c2pool = ctx.enter_context(tc.tile_pool(name="c2", bufs=2))
            rpool = ctx.enter_context(tc.tile_pool(name="r1p", bufs=12))
            upool = ctx.enter_context(tc.tile_pool(name="uu", bufs=3))
            utpool = ctx.enter_context(tc.tile_pool(name="utp", bufs=3))
            v2pool = ctx.enter_context(tc.tile_pool(name="v2p", bufs=4))
            spool = ctx.enter_context(tc.tile_pool(name="scr", bufs=3))
            psA = ctx.enter_context(
                tc.tile_pool(name="psA", bufs=6, space=bass.MemorySpace.PSUM))
            psT = ctx.enter_context(
                tc.tile_pool(name="psT", bufs=2, space=bass.MemorySpace.PSUM))

            for _rep in range(nrep):
                def load(ap, shape, tag, dt=bf16, pool=None):
                    t = (pool or c2pool).tile(shape, dt, tag=tag, name=tag)
                    nc.sync.dma_start(t[:], ap[:])
                    return t

                t_gxE = load(gxE, [81, 16 * NE], "gxE", pool=cpool)
                t_Wblk = load(Wblk, [81, 256], "Wblk")
                t_St = load(St, [128, 8, 64], "St", fp8)
                t_idB = load(idB, [128, 64], "idB")
                t_xT2c = load(xT2c, [17, 32 * 64], "xT2c")
                t_Wla4c = load(Wla4c, [17, 128], "Wla4c")
                t_Wlb4c = load(Wlb4c, [17, 128], "Wlb4c")
                t_WlapB = load(WlapB, [128, 128], "WlapB")
                t_WlbpB = load(WlbpB, [128, 128], "WlbpB")
                t_G2t = load(G2t, [128, E2 + 13], "G2t")
                t_selD = load(selD, [2, 2, 128], "selD", fp8)
                t_selF4 = load(selF4, [4, 128], "selF4")
                t_selC8 = load(selC8, [8, 128], "selC8")
                t_actD0 = load(actD0, [2, 2, 16 * NE], "actD0", fp8, pool=cpool)
                t_actD1 = load(actD1, [2, 2, 16 * NE], "actD1", fp8, pool=cpool)
                t_actF = load(actF, [8, 48], "actF")
                t_blc = load(blc, [128, 1], "blc", f32)
                t_WvP = load(WvP, [128, 4], "WvP", f32)
                t_WvPb = load(WvPb, [128, 4], "WvPb")
                t_S1 = accp.tile([128, 64], f32, tag="S1", name="S1")
                t_z = accp.tile([128, 512], f32, tag="zz", name="zz")
                nc.gpsimd.memset(t_z[:], 0.0)
                t_factS = accp.tile([128, 96], bf16, tag="factS", name="factS")
                nc.gpsimd.memset(t_factS[:], 0.0)

                def ph2(p):
                    """Edge MLP layer1 + relu for both 8g-groups of pair p."""
                    r1s = {0: [], 1: []}
                    for cc in range(4):
                        for gi, g8 in ((0, 2 * p), (1, 2 * p + 1)):
                            p1 = psA.tile([128, 512], f32, tag="A", name="p1")
                            for h in range(2):
                                c = 2 * cc + h
                                off = g8 * NE + c * 128
                                nc.tensor.matmul(
                                    p1[:, h * 256:(h + 1) * 256],
                                    t_gxE[:, off:off + 128], t_Wblk[:],
                                    start=True, stop=True)
                            # r1 holds chunk 2cc in j=0 plane, 2cc+1 in j=1
                            r = rpool.tile([128, 2, 256], fp8, tag="r1", name="r1")
                            if (2 * cc + gi) % 8 not in (1, 4, 7):
                                nc.scalar.activation(r[:, :, :], p1[:], RELU)
                            else:
                                nc.vector.tensor_scalar_max(r[:, :, :], p1[:], 0.0)
                            r1s[gi].append(r)
                    return r1s

                def ph3(p, r1s):
                    """DoubleRow fp8 scatter burst + U evacuation."""
                    # both groups at partition base 0 (DoubleRow + base-64 out
                    # fails the ISA check): group 1 in free cols 256:512 rides
                    # group 0's region-open (start would re-zero the 2KB bank)
                    pu = psA.tile([128, 512], f32, tag="A", name="pu")
                    for cc in range(4):
                        stc = t_St[:, 2 * cc:2 * cc + 2, :]
                        nc.tensor.matmul(pu[0:64, 0:256], stc, r1s[0][cc][:, :, :],
                                         start=(cc == 0), stop=(cc == 3),
                                         perf_mode=DR)
                        nc.tensor.matmul(pu[0:64, 256:512], stc, r1s[1][cc][:, :, :],
                                         start=False, stop=(cc == 3),
                                         perf_mode=DR, skip_group_check=True)
                    t_U = upool.tile([64, 512], bf16, tag="UU", name="UU")
                    if p % 2 == 0:
                        nc.scalar.copy(t_U[0:64, :], pu[0:64, 0:512])
                    else:
                        nc.vector.tensor_copy(t_U[0:64, :], pu[0:64, 0:512])
                    return t_U

                def tail(p, t_U):
                    """ph4 transposes, ph5 V2 tables, ph6 critic head for pair p."""
                    pt = psT.tile([128, 256], bf16, tag="T", name="pt")
                    for k in range(4):
                        src = t_U[0:64, k * 128:(k + 1) * 128]
                        idn = t_idB[0:64, :]
                        nc.tensor.transpose(pt[:, k * 64:(k + 1) * 64], src, idn)
                    t_UT = utpool.tile([128, 256], bf16, tag="UT", name="UT")
                    nc.vector.tensor_copy(t_UT[:], pt[:])

                    v2t = []
                    for h in range(2):
                        pv = psA.tile([128, 512], f32, tag="A", name="pv")
                        for k2 in range(2):
                            k = 2 * h + k2
                            s = 4 * p + k
                            col = slice(k2 * 128, (k2 + 1) * 128)
                            lx = t_xT2c[:, s * 64:(s + 1) * 64]
                            ut = t_UT[:, k * 64:(k + 1) * 64]
                            nc.tensor.matmul(pv[0:64, col], lx, t_Wla4c[:],
                                             start=True, stop=False)
                            nc.tensor.matmul(pv[0:64, col], ut, t_WlapB[:],
                                             start=False, stop=True)
                            nc.tensor.matmul(pv[64:128, col], lx, t_Wlb4c[:],
                                             start=True, stop=False,
                                             skip_group_check=True)
                            nc.tensor.matmul(pv[64:128, col], ut, t_WlbpB[:],
                                             start=False, stop=True,
                                             skip_group_check=True)
                        v2 = v2pool.tile([128, 256], bf16, tag="v2", name="v2")
                        if h == 0:
                            nc.scalar.copy(v2[:], pv[:, 0:256])
                        else:
                            nc.vector.tensor_copy(v2[:], pv[:, 0:256])
                        v2t.append(v2)

                    factP = psA.tile([128, 512], f32, tag="A", name="factP")
                    zeroed = False
                    for h in range(2):
                        q = 2 * p + h
                        v2 = v2t[h]
                        p6s = []
                        for si in range(2):
                            fcol = slice((2 * h + si) * 3, (2 * h + si) * 3 + 3)
                            vsl = v2[:, si * 128:(si + 1) * 128]
                            pA6 = psA.tile([128, 512], f32, tag="A", name="pA6")
                            pB6 = psA.tile([128, 512], f32, tag="A", name="pB6")
                            nc.tensor.matmul(pA6[:], vsl, t_G2t[:, 0:512],
                                             start=True, stop=False)
                            nc.tensor.matmul(pB6[:], vsl, t_G2t[:, 512:1024],
                                             start=True, stop=False)
                            if no_fact:
                                p6s.append((pA6, pB6))
                                continue
                            if not zeroed:
                                # zeroing matmul opens the region and orders all
                                # factory matmuls after it via write hazards
                                nc.tensor.matmul(factP[:, 0:12], vsl,
                                                 t_G2t[:, E2 + 1:E2 + 13],
                                                 start=True, stop=False)
                                zeroed = True
                            nc.tensor.matmul(factP[:, fcol], vsl, t_G2t[:, 1024:1027],
                                             start=False, stop=False,
                                             skip_group_check=True)
                            p6s.append((pA6, pB6))
                        # action matmuls: fp8 DoubleRow, K=2x2 (graph g=2k+j)
                        for blk in range(3):
                            for si in range(2):
                                pA6, pB6 = p6s[si]
                                s = 2 * q + si       # 4-graph subgroup 0..31
                                if blk == 0:
                                    dst, a0 = pA6[:], 0
                                elif blk == 1:
                                    dst, a0 = pB6[:], 512
                                else:
                                    if no_fact:
                                        continue
                                    # factory act MMs (bf16, tiny N=3)
                                    fcol = slice((2 * h + si) * 3, (2 * h + si) * 3 + 3)
                                    dst = factP[:, fcol]
                                    if si == 0:
                                        selc = t_selF4[0:4, :]
                                        arow = t_actF[0:4, 3 * q:3 * q + 3]
                                    else:
                                        selc = t_selC8[0:8, :]
                                        arow = t_actF[0:8, 3 * q:3 * q + 3]
                                    nc.tensor.matmul(dst, selc, arow,
                                                     start=False, stop=False,
                                                     skip_group_check=True)
                                    continue
                                asrc = t_actD0 if s < 16 else t_actD1
                                so = (s % 16) * NE + a0
                                arow = asrc[:, :, so:so + 512]
                                nc.tensor.matmul(dst, t_selD[:, :, :], arow,
                                                 start=False, stop=True,
                                                 perf_mode=DR)
                        for si in range(2):
                            s = 2 * q + si
                            m = 2 * h + si
                            pA6, pB6 = p6s[si]
                            scrA = spool.tile([128, 512], bf16, tag="scrA", name="scrA")
                            scrB = spool.tile([128, 512], bf16, tag="scrB", name="scrB")
                            nc.scalar.activation(
                                scrA[:], pA6[:], RELU, bias=t_blc[:],
                                accum_out=t_S1[:, 2 * s:2 * s + 1])
                            nc.vector.scalar_tensor_tensor(
                                scrB[:], pB6[:], t_blc[:], t_z[:],
                                ADD, MAX,
                                accum_out=t_S1[:, 2 * s + 1:2 * s + 2])
                    if not no_fact:
                        # closing no-op accumulate stops the factP group
                        nc.tensor.matmul(factP[:, 0:12], v2t[1][:, 0:128],
                                         t_G2t[:, E2 + 1:E2 + 13],
                                         start=False, stop=True)
                        # factory columns: relu once per pair into factS
                        nc.vector.tensor_scalar(
                            t_factS[:, p * 12:(p + 1) * 12], factP[:, 0:12],
                            t_blc[:], 0.0, ADD, MAX)

                # software pipeline: pair p's MLP overlaps pair p-1's tail
                prevU = None
                for p in range(NPAIR):
                    r1s = ph2(p)
                    if prevU is not None:
                        tail(p - 1, prevU)
                    prevU = ph3(p, r1s)
                tail(NPAIR - 1, prevU)

                # ---- finale ----
                pf = psA.tile([128, 512], f32, tag="A", name="pf")
                nc.tensor.matmul(pf[0:4, 0:64], t_WvP[:], t_S1[:],
                                 start=True, stop=True)
                nc.tensor.matmul(pf[0:4, 64:160], t_WvPb[:], t_factS[:],
                                 start=True, stop=True)
                fo = cpool.tile([4, 160], f32, tag="fo", name="fo")
                nc.vector.tensor_copy(fo[:], pf[0:4, 0:160])
                nc.sync.dma_start(vout[:], fo[:])

    nc.compile()
    return nc


def _host_prep(inputs):
    x = np.asarray(inputs["x"], np.float32).reshape(B, NN, NODE)
    ea = np.asarray(inputs["edge_attr"], np.float32).reshape(B, NE, EDGEF)
    act = np.asarray(inputs["action"], np.float32)
    es = np.asarray(inputs["edges_src"]).astype(np.int64)
    ed = np.asarray(inputs["edges_dst"]).astype(np.int64)
    W1 = np.asarray(inputs["W1"], np.float32)
    b1 = np.asarray(inputs["b1"], np.float32)
    W2 = np.asarray(inputs["W2"], np.float32)
    b2 = np.asarray(inputs["b2"], np.float32)
    Wl = np.asarray(inputs["Wl"], np.float32)
    bl = np.asarray(inputs["bl"], np.float32)
    Wv = np.asarray(inputs["Wv"], np.float32)
    bv = np.asarray(inputs["bv"], np.float32)

    Wlap = W2 @ Wl[4:36]
    Wlbp = W2 @ Wl[40:72]
    wlc = Wl[72]
    cn = np.bincount(es, minlength=NN).astype(np.float32)
    corr_a = b2 @ Wlap
    corr_b = b2 @ Wlbp

    consts = {}
    Wblk = np.zeros((81, 256), np.float32)
    for gl in range(8):
        sl = slice(32 * gl, 32 * gl + 32)
        Wblk[8 * gl:8 * gl + 4, sl] = W1[0:4]
        Wblk[8 * gl + 4:8 * gl + 8, sl] = W1[4:8]
        Wblk[64 + 2 * gl:64 + 2 * gl + 2, sl] = W1[8:10]
        Wblk[80, sl] = b1
    consts["Wblk"] = Wblk.astype(BF)

    St = np.zeros((128, 8 * 64), np.float32)
    for c in range(8):
        St[np.arange(128), c * 64 + es[c * 128:(c + 1) * 128]] = 1.0
    consts["St"] = St.reshape(128, 8, 64).astype(F8)

    idB = np.zeros((128, 64), np.float32)
    idB[0:64] = np.eye(64)
    idB[64:128] = np.eye(64)
    consts["idB"] = idB.astype(BF)

    Wla4c = np.zeros((17, 128), np.float32)
    Wlb4c = np.zeros((17, 128), np.float32)
    WlapB = np.zeros((128, 128), np.float32)
    WlbpB = np.zeros((128, 128), np.float32)
    for gl in range(4):
        sl = slice(32 * gl, 32 * gl + 32)
        Wla4c[4 * gl:4 * gl + 4, sl] = Wl[0:4]
        Wlb4c[4 * gl:4 * gl + 4, sl] = Wl[36:40]
        Wla4c[16, sl] = corr_a
        Wlb4c[16, sl] = corr_b
        WlapB[sl, sl] = Wlap
        WlbpB[sl, sl] = Wlbp
    consts["Wla4c"] = Wla4c.astype(BF)
    consts["Wlb4c"] = Wlb4c.astype(BF)
    consts["WlapB"] = WlapB.astype(BF)
    consts["WlbpB"] = WlbpB.astype(BF)

    G2t = np.zeros((128, E2 + 13), np.float32)
    G2t[es, np.arange(NE)] = 1.0
    G2t[64 + ed, np.arange(NE)] += 1.0
    for i in range(NFACT):
        G2t[61 + i, NE + i] = 1.0
        G2t[64 + 61 + i, NE + i] = 1.0
    consts["G2t"] = G2t.astype(BF)

    selD = np.zeros((2, 2, 128), np.float32)
    for k in range(2):
        for j in range(2):
            g = 2 * k + j
            selD[k, j, 32 * g:32 * g + 32] = wlc
    consts["selD"] = selD.astype(F8)
    selF4 = np.zeros((4, 128), np.float32)
    for k in range(4):
        selF4[k, 32 * k:32 * k + 32] = wlc
    consts["selF4"] = selF4.astype(BF)
    selC8 = np.zeros((8, 128), np.float32)
    for k in range(4):
        selC8[4 + k, 32 * k:32 * k + 32] = wlc
    consts["selC8"] = selC8.astype(BF)

    blcol = np.zeros((128, 1), np.float32)
    WvP = np.zeros((128, 4), np.float32)
    for gl in range(4):
        blcol[32 * gl:32 * gl + 32, 0] = bl
        WvP[32 * gl:32 * gl + 32, gl] = Wv[:, 0]
    consts["blc"] = blcol
    consts["WvP"] = WvP
    consts["WvPb"] = WvP.astype(BF)

    in_maps = []
    for t in range(NCORES):
        m = dict(consts)
        xs = x[t * GPC:(t + 1) * GPC]            # [128, 64, 4]
        eas = ea[t * GPC:(t + 1) * GPC]          # [128, 1024, 2]
        acs = act[t * GPC:(t + 1) * GPC]         # [128, 1027]

        gxs = xs[:, es, :]                       # [128, 1024, 4]
        gxd = xs[:, ed, :]
        rows = np.concatenate([gxs.transpose(0, 2, 1),
                               gxd.transpose(0, 2, 1)], axis=1)  # [128, 8, 1024]
        gxE = np.empty((81, 16 * NE), np.float32)
        gxE[0:64] = rows.reshape(16, 64, NE).transpose(1, 0, 2).reshape(64, 16 * NE)
        erows = eas.transpose(0, 2, 1).reshape(16, 16, NE)
        gxE[64:80] = erows.transpose(1, 0, 2).reshape(16, 16 * NE)
        gxE[80] = 1.0
        m["gxE"] = np.ascontiguousarray(gxE).astype(BF)

        xT2c = np.empty((17, 32 * 64), np.float32)
        xt = xs.reshape(32, 4, NN, NODE).transpose(0, 1, 3, 2)
        xT2c[0:16] = xt.reshape(32, 16, NN).transpose(1, 0, 2).reshape(16, 32 * NN)
        xT2c[16] = np.tile(cn, 32)
        m["xT2c"] = xT2c.astype(BF)

        a4 = acs.reshape(32, 4, E2)
        # edge actions: [k, j, s*NE + e] = act[graph 4s + 2k+j, e]
        ae = a4[:, :, 0:NE]                      # [32, 4, 1024]
        actD = np.empty((2, 2, 32 * NE), np.float32)
        for k in range(2):
            for j in range(2):
                actD[k, j] = ae[:, 2 * k + j, :].reshape(-1)
        a8 = actD.astype(F8)
        m["actD0"] = a8[:, :, :16 * NE].copy()
        m["actD1"] = a8[:, :, 16 * NE:].copy()
        # factory actions: rows 0:4 = even subgroup 2q, rows 4:8 = odd 2q+1
        af = a4[:, :, NE:E2]                     # [32, 4, 3]
        actF = np.zeros((8, 48), np.float32)
        actF[0:4] = af[0::2].transpose(1, 0, 2).reshape(4, 48)
        actF[4:8] = af[1::2].transpose(1, 0, 2).reshape(4, 48)
        m["actF"] = actF.astype(BF)
        in_maps.append(m)

    extra = float(E2) * float(bv.reshape(-1)[0])
    return in_maps, extra


def _assemble(results, extra):
    out = np.empty((B,), np.float32)
    for t in range(NCORES):
        v = results[t]["v"]                      # [4, 160]
        per = v[:, 0:64:2] + v[:, 1:64:2]        # [4, 32]
        facts = v[:, 64:160].reshape(4, 32, 3).sum(-1)
        out[t * GPC:(t + 1) * GPC] = (per + facts).T.reshape(-1) + extra
    return out


def kernel(**inputs) -> np.ndarray:
    if "nc" not in _CACHE:
        _CACHE["nc"] = _build_nc()
    nc = _CACHE["nc"]
    in_maps, extra = _host_prep(inputs)
    res = bass_utils.run_bass_kernel_spmd(nc, in_maps, list(range(NCORES)))
    return _assemble(res.results, extra)

